# revision 76
# baseline (speedup 1.0000x reference)
"""Trainium2 Bass kernel for nn_NetFV (NetFV pooling head).

Strategy (pure data parallel over 8 cores, 256 batches each):
  - Host prep (bf16): x in two layouts — natural superbatch-packed and
    transposed double-stacked [122, 2432] per superbatch (rows 0:61 =
    [x^T; ones] of batches 0-3, 61:122 of batches 4-7, padded to 128
    partitions). Per-(f,c) finishing constants folded into 128-row
    blocks; head weights rearranged; 2nd l2-normalize of fv1 folded in.
  - DMA: one transfer per 2-superbatch granule per stream, ONE big
    (~19KB) descriptor per partition. The HWDGE splits a transfer over
    E = (largest divisor of the partition count <= 16) DMA engines, so
    xt is padded to 128 partitions; each descriptor costs ~70ns +
    bytes/22.5GBps on its engine. The two streams go out on the two
    HWDGE rings (nc.sync + nc.scalar) to transfer concurrently.
  - Per superbatch of 8 batches:
      pack: x copied (DVE) into 40 [120,128] chunks (cols 0:60), x^2
      (split DVE/ACT) into 64:124, constant ones col 124.
      logits: 20 matmuls lhsT=xtt window [122,128], rhs=waug2 [122,16]
      (both halves' W stacked with exact zeros) -> 2x fewer weight
      loads; softmax: exp (ACT) / rowsum+recip (DVE) / scale (GPSIMD).
      fv: per batch 5 accumulating matmuls lhsT=xaug chunk [120,128],
      rhs=act [120,8] -> psum [128,8] = fv1(0:60)|fv2(64:124)|asum(124).
  - Software pipelining: all engine queues are strict in-order, so
    emission order is scheduled by readiness: softmax+fv of sb-2, then
    pack+logits of sb-1, with granule DMAs issued 1-2 superbatches
    ahead and per-64-batch finishing deferred 4 superbatches so its
    cross-engine chain never blocks queue heads.
  - Finishing per 64 batches: elementwise work fused over fv1+fv2 as
    [128, 512] DVE ops with row-block constants; partition reductions
    and broadcasts via small PE matmuls; head as 16 accumulated
    [60,64]x[60,18] matmuls.
"""

import math
import sys

for _p in ("/opt/trn_rl_repo", "/opt/pypackages"):
    if _p not in sys.path:
        sys.path.append(_p)

import ml_dtypes
import numpy as np

import concourse.bacc as bacc
import concourse.bass as bass
import concourse.mybir as mybir
import concourse.tile as tile
from concourse.bass_utils import run_bass_kernel_spmd

F, M, C, OUT = 60, 600, 8, 18
B = 2048
NCORES = 8
BL = B // NCORES            # 256 batches per core
SB = 8                      # batches per superbatch
NSB = BL // SB              # 32 superbatches
FGB = 32                    # batches per finishing group
NFG = BL // FGB             # 4 finishing groups
SBPF = FGB // SB            # 8 superbatches per finishing group
CH = 5                      # chunks (of 120 rows) per batch
RP = M // CH                # 120 rows per chunk
HW2 = 2432                  # padded half width (2400 + 32)
NW = 20                     # logit windows per superbatch
NG = FGB * C                # 512 finishing columns

BF16 = mybir.dt.bfloat16
F8 = mybir.dt.float8e4
F32 = mybir.dt.float32
MULT = mybir.AluOpType.mult
EPS = 1e-12


def _build_nc():
    nc = bacc.Bacc(
        "TRN2", target_bir_lowering=False, debug=False,
        enable_asserts=False, num_devices=NCORES,
    )
    # DMA granularity: one transfer per 4 superbatches, ONE ~19.5KB
    # descriptor per partition. The HWDGE splits a transfer across
    # E = (largest divisor of the partition count <= 16) DMA engines,
    # each engine serving partition_count/E contiguous partitions, and
    # each descriptor costs ~70ns + bytes/22.5GBps on its engine. So:
    # xt is padded to 128 partitions (-> 16 engines; 122 -> only 2!),
    # xg keeps 120 (-> 15 engines), and descriptors are made as large
    # as possible (one per partition per granule).
    xg = nc.dram_tensor("xg", [NSB // 2, RP, 2 * SB * CH * F], BF16,
                        kind="ExternalInput").ap()
    xt = nc.dram_tensor("xt", [NSB // 2, 128, 2 * HW2], BF16,
                        kind="ExternalInput").ap()
    waug2_d = nc.dram_tensor("waug2", [128, 2 * C], BF16,
                             kind="ExternalInput").ap()
    cst_d = nc.dram_tensor("cst", [128, 3 * C], F32, kind="ExternalInput").ap()
    sel2_d = nc.dram_tensor("sel2", [33, 128], F32, kind="ExternalInput").ap()
    ones2_d = nc.dram_tensor("ones2", [124, 33], F32, kind="ExternalInput").ap()
    sela_d = nc.dram_tensor("sela", [125, 128], F32, kind="ExternalInput").ap()
    hds_d = nc.dram_tensor("hds", [F, 2 * C * OUT], F32, kind="ExternalInput").ap()
    y = nc.dram_tensor("y", [BL, OUT], F32, kind="ExternalOutput").ap()

    with tile.TileContext(nc) as tc:
        _emit(tc, y, xg, xt, waug2_d, cst_d, sel2_d, ones2_d, sela_d, hds_d)
    nc.compile()
    return nc


def _emit(tc, y, xg, xt, waug2_d, cst_d, sel2_d, ones2_d, sela_d, hds_d):
    nc = tc.nc
    from contextlib import ExitStack
    ctx = ExitStack()
    with ctx:
        cpool = ctx.enter_context(tc.tile_pool(name="cpool", bufs=1))
        xnpool = ctx.enter_context(tc.tile_pool(name="xnpool", bufs=2))
        tpool = ctx.enter_context(tc.tile_pool(name="tpool", bufs=2))
        spool = ctx.enter_context(tc.tile_pool(name="spool", bufs=3))
        gpool = ctx.enter_context(tc.tile_pool(name="gpool", bufs=2))
        fpool = ctx.enter_context(tc.tile_pool(name="fpool", bufs=2))
        lpsum = ctx.enter_context(tc.tile_pool(name="lpsum", bufs=3, space="PSUM"))
        fpsum = ctx.enter_context(tc.tile_pool(name="fpsum", bufs=3, space="PSUM"))
        # asb/r12/nb/hp have strictly sequential lifetimes: one rotating slot
        finp = ctx.enter_context(tc.tile_pool(name="finp", bufs=2, space="PSUM"))

        # ---- constants ----
        waug2 = cpool.tile([128, 2 * C], BF16)
        nc.scalar.dma_start(out=waug2[:], in_=waug2_d[:])
        cst = cpool.tile([128, 3 * C], F32)
        nc.scalar.dma_start(out=cst[:], in_=cst_d[:])
        hds = cpool.tile([F, 2 * C * OUT], F32)
        nc.scalar.dma_start(out=hds[:], in_=hds_d[:])
        cstA = cst[:, 0 * C:1 * C]        # rows 0:60 = 1/cw, 64:124 = 1/cw^2
        cstB = cst[:, 1 * C:2 * C]        # rows 0:60 = w2/cw, 64:124 = 1 - w2^2/cw^2
        cstC = cst[0:F, 2 * C:3 * C]      # rows 0:60 = 2*w2/cw^2

        ones2 = cpool.tile([124, 33], F32)    # lhsT: col0 sums rows 0:60,
        nc.scalar.dma_start(out=ones2[:], in_=ones2_d[:])  # col32 rows 64:124
        sel2 = cpool.tile([33, 128], F32)     # lhsT: row-block select for norms
        nc.scalar.dma_start(out=sel2[:], in_=sel2_d[:])
        sela = cpool.tile([125, 128], F32)    # lhsT: broadcast stage row 124
        nc.scalar.dma_start(out=sela[:], in_=sela_d[:])
        eps1 = cpool.tile([1, 1], F32)        # l2-normalize epsilon
        nc.vector.memset(eps1[:], EPS)


        def cb(ap, p):  # broadcast a [p, C] const across FGB batches
            return ap.unsqueeze(1).broadcast_to([p, FGB, C])

        # xaug tiles: 3 fixed buffers rotated manually so the constant
        # columns (zeros 60:64, ones 124, zeros 125:128) are written once
        xaugs = []
        for i in range(3):
            xa = cpool.tile([RP, SB * CH * 128], BF16, tag=f"xaug{i}")
            xav = xa.rearrange("p (k q) -> p k q", q=128)
            nc.vector.memset(xav[:, :, 60:64], 0.0)
            nc.vector.memset(xav[:, :, 124:125], 1.0)
            nc.vector.memset(xav[:, :, 125:128], 0.0)
            xaugs.append(xa)

        # Software pipelining: the PE queue is strict in-order, so emission
        # order decides what PE work is available while waiting on other
        # engines. Emit logits(sb) before softmax+fv(sb-1), and defer each
        # group's finishing by 2 superbatches of PE work.
        cur = {}       # granule tiles
        gran = {}      # g2 -> granule tiles
        sbst = {}      # sb -> (lp, xgt)
        stages = {}    # fg -> stage tile

        def issue_dma(sb):
            g2 = sb // 2
            xnt = xnpool.tile([RP, 2 * SB * CH * F], BF16, tag="xnt", name="xnt")
            xtt4 = tpool.tile([128, 2 * HW2], BF16, tag="xtt4", name="xtt4")
            nc.sync.dma_start(out=xnt[:], in_=xg[g2])
            # second HWDGE ring (ACT): xg/xt transfers run concurrently
            nc.scalar.dma_start(out=xtt4[:], in_=xt[g2])
            gran[g2] = (xnt, xtt4)

        def pack_logits(sb):
            si = sb % 2
            xnt, xtt4 = gran[sb // 2]
            if si == 1:
                gran.pop(sb // 2)
            # pack superbatch: x into cols 0:60, x^2 into 64:124
            xgt = xaugs[sb % 3]
            xgv = xgt.rearrange("p (k q) -> p k q", q=128)
            xnv = xnt.rearrange("p (s k f) -> p s k f", s=2, f=F)
            nc.vector.tensor_copy(xgv[:, :, 0:F], xnv[:, si])
            nc.vector.tensor_tensor(
                out=xgv[:, 0:20, 64:64 + F], in0=xnv[:, si, 0:20],
                in1=xnv[:, si, 0:20], op=MULT,
            )
            nc.scalar.activation(
                xgv[:, 20:40, 64:64 + F], xnv[:, si, 20:40],
                mybir.ActivationFunctionType.Square,
            )
            xtt = xtt4.rearrange("p (s q) -> p s q", s=2)[:, si]
            # logits: stacked halves, 20 windows x [128, 16]
            lp = lpsum.tile([128, NW * 2 * C], F32)
            for w in range(NW):
                nc.tensor.matmul(
                    lp[:, w * 2 * C:(w + 1) * 2 * C],
                    xtt[:, RP * w: RP * w + 128],   # [128, 128] K=128
                    waug2[:],
                    start=True, stop=True,
                )
            sbst[sb] = (lp, xgt)

        def stage_b(sb):
            fg, s = divmod(sb, SBPF)
            if s == 0:
                stages[fg] = gpool.tile([128, NG], F32, tag="stage", name="stage")
            stage = stages[fg]
            lp, xgt = sbst.pop(sb)
            # softmax over C
            expt = spool.tile([RP, NW * 2 * C], F32, tag="expt")
            nc.scalar.activation(
                expt[:], lp[0:RP, :], mybir.ActivationFunctionType.Exp
            )
            sums = spool.tile([RP, NW * 2], F32, tag="sums")
            nc.vector.reduce_sum(
                out=sums[:],
                in_=expt.rearrange("p (k e) -> p k e", e=C),
                axis=mybir.AxisListType.X,
            )
            rin = spool.tile([RP, NW * 2], F32, tag="rin")
            nc.vector.reciprocal(rin[:], sums[:])
            actt = spool.tile([RP, NW * 2 * C], BF16, tag="actt")
            nc.gpsimd.tensor_tensor(
                out=actt.rearrange("p (k e) -> p k e", e=C),
                in0=expt.rearrange("p (k e) -> p k e", e=C),
                in1=rin.unsqueeze(2).broadcast_to([RP, NW * 2, C]),
                op=MULT,
            )
            # fv accumulation; actt col block (w, h): batch 4h+w//5, chunk w%5
            fp = fpsum.tile([128, SB * C], F32)
            for b in range(SB):
                h, wb = divmod(b, 4)
                for c5 in range(CH):
                    w = wb * CH + c5
                    nc.tensor.matmul(
                        fp[:, b * C:(b + 1) * C],
                        xgt[:, (b * CH + c5) * 128:(b * CH + c5 + 1) * 128],
                        actt[:, (w * 2 + h) * C:(w * 2 + h + 1) * C],
                        start=(c5 == 0), stop=(c5 == CH - 1),
                    )
            nc.scalar.copy(
                stage[:, s * SB * C:(s + 1) * SB * C], fp[:]
            )

        def finishing(fg):
            stage = stages.pop(fg)
            # ---- finishing for this group of 64 batches ----
            # fused row-blocks: rows 0:60 = fv1 path, 64:124 = fv2 path
            # asb[p, n] = stage[124, n] (asum) broadcast to all partitions
            asb = finp.tile([128, NG], F32, tag="fin")
            nc.tensor.matmul(asb[:], sela[:], stage[0:125, :],
                             start=True, stop=True)

            t1 = fpool.tile([128, NG], F32, tag="t1")
            nc.vector.tensor_tensor(out=t1.rearrange("p (g e) -> p g e", e=C),
                                    in0=stage.rearrange("p (g e) -> p g e", e=C),
                                    in1=cb(cstA, 128), op=MULT)
            m1 = fpool.tile([128, NG], F32, tag="m1")
            nc.vector.tensor_tensor(out=m1.rearrange("p (g e) -> p g e", e=C),
                                    in0=asb.rearrange("p (g e) -> p g e", e=C),
                                    in1=cb(cstB, 128), op=MULT)
            fvn = fpool.tile([128, NG], F32, tag="fvn")
            nc.vector.tensor_sub(fvn[:], t1[:], m1[:])
            u4 = fpool.tile([64 + F, NG], F32, tag="u4")
            nc.vector.tensor_tensor(out=u4[64:64 + F, :].rearrange("p (g e) -> p g e", e=C),
                                    in0=stage[0:F].rearrange("p (g e) -> p g e", e=C),
                                    in1=cb(cstC, F), op=MULT)
            nc.vector.tensor_sub(fvn[64:64 + F, :], fvn[64:64 + F, :],
                                 u4[64:64 + F, :])
            q1 = fpool.tile([128, NG], F32, tag="q1")
            nc.vector.tensor_mul(q1[:], fvn[:], fvn[:])
            r12 = finp.tile([33, NG], F32, tag="fin")
            nc.tensor.matmul(r12[:], ones2[:], q1[0:124, :], start=True, stop=True)

            nrB = fpool.tile([33, NG], F32, tag="nrB")
            nc.vector.memset(nrB[:], 0.0)
            sq1 = fpool.tile([1, NG], F32, tag="sq1")
            nc.scalar.activation(sq1[:], r12[0:1, :],
                                 mybir.ActivationFunctionType.Sqrt, bias=eps1[:])
            nc.vector.reciprocal(nrB[0:1, :], sq1[:])
            r2c = fpool.tile([1, FGB], F32, tag="r2c")
            nc.vector.reduce_sum(out=r2c[:],
                                 in_=r12[32:33, :].rearrange("p (g e) -> p g e", e=C),
                                 axis=mybir.AxisListType.X)
            sq2 = fpool.tile([1, FGB], F32, tag="sq2")
            nc.scalar.activation(sq2[:], r2c[:],
                                 mybir.ActivationFunctionType.Sqrt, bias=eps1[:])
            nr2 = fpool.tile([1, FGB], F32, tag="nr2")
            nc.vector.reciprocal(nr2[:], sq2[:])
            nc.vector.tensor_copy(
                nrB[32:33, :].rearrange("p (g e) -> p g e", e=C),
                nr2.unsqueeze(2).broadcast_to([1, FGB, C]),
            )
            nb = finp.tile([128, NG], F32, tag="fin")
            nc.tensor.matmul(nb[:], sel2[:], nrB[:], start=True, stop=True)
            fvnn = fpool.tile([128, NG], F32, tag="fvnn")
            nc.vector.tensor_mul(fvnn[:], fvn[:], nb[:])
            fv2c = fpool.tile([F, NG], F32, tag="fv2c")
            nc.vector.tensor_copy(fv2c[:], fvnn[64:64 + F, :])

            # ---- head ----
            hp = finp.tile([FGB, OUT], F32, tag="fin")
            for ci in range(C):
                nc.tensor.matmul(
                    hp[:], fvnn[0:F, ci::C], hds[:, ci * OUT:(ci + 1) * OUT],
                    start=(ci == 0), stop=False,
                )
            for ci in range(C):
                nc.tensor.matmul(
                    hp[:], fv2c[:, ci::C],
                    hds[:, (C + ci) * OUT:(C + ci + 1) * OUT],
                    start=False, stop=(ci == C - 1),
                )
            yt = fpool.tile([FGB, OUT], F32, tag="yt")
            nc.scalar.copy(yt[:], hp[:])
            nc.sync.dma_start(out=y[fg * FGB:(fg + 1) * FGB, :], in_=yt[:])

        for t in range(NSB + 2):
            if t < NSB and t % 2 == 0:
                issue_dma(t)
            if t >= 2:
                stage_b(t - 2)
            if 1 <= t <= NSB:
                pack_logits(t - 1)
            if t >= SBPF + 5 and (t - SBPF - 5) % SBPF == 0:
                finishing((t - SBPF - 5) // SBPF)
        finishing(NFG - 1)


def _host_prep(reshaped_input, cluster_weights, covar_weights, cluster_biases,
               cluster_weights2, hidden1_weights):
    bf = ml_dtypes.bfloat16
    x = np.ascontiguousarray(reshaped_input, dtype=np.float32)
    xb = x.astype(bf)                                   # [B*M, F]
    # natural superbatch-packed, 4-sb granules, one row per partition
    xgp = (xb.reshape(NCORES, NSB, SB * CH, RP, F)
             .transpose(0, 1, 3, 2, 4)
             .reshape(NCORES, NSB // 2, 2, RP, 2400)
             .transpose(0, 1, 3, 2, 4)
             .reshape(NCORES, NSB // 2, RP, 2 * 2400))
    xgp = np.ascontiguousarray(xgp)
    # transposed double-stacked: [core, sb, 2*(F+1), HW2]
    #   rows 0:60 = x^T of batches 0-3 concat, row 60 = ones
    #   rows 61:121 = x^T of batches 4-7, row 121 = ones
    x6 = (xb.reshape(NCORES, NSB, 2, 4 * M, F)
            .transpose(0, 1, 2, 4, 3))                  # [NC, NSB, 2, F, 2400]
    xtp = np.zeros((NCORES, NSB, 2, F + 1, HW2), dtype=bf)
    xtp[:, :, :, :F, :4 * M] = x6
    xtp[:, :, :, F, :] = bf(1.0)
    # 4-sb granules, one row per partition, padded to 128 partitions
    xtp2 = np.zeros((NCORES, NSB // 2, 128, 2 * HW2), dtype=bf)
    xtp2[:, :, :2 * (F + 1), :] = (
        xtp.reshape(NCORES, NSB // 2, 2, 2 * (F + 1), HW2)
           .transpose(0, 1, 3, 2, 4)
           .reshape(NCORES, NSB // 2, 2 * (F + 1), 2 * HW2))
    xtp = xtp2

    waug2 = np.zeros((128, 2 * C), dtype=bf)
    waug2[0:F, 0:C] = cluster_weights.astype(bf)
    waug2[F, 0:C] = cluster_biases.astype(bf)
    waug2[F + 1:2 * F + 1, C:2 * C] = cluster_weights.astype(bf)
    waug2[2 * F + 1, C:2 * C] = cluster_biases.astype(bf)

    cw = np.square(covar_weights.astype(np.float64)) + 1e-6       # [F, C]
    w2 = cluster_weights2[0].astype(np.float64)                   # [F, C]
    cst = np.zeros((128, 3 * C), dtype=np.float32)
    cst[0:F, 0 * C:1 * C] = 1.0 / cw
    cst[64:64 + F, 0 * C:1 * C] = 1.0 / np.square(cw)
    cst[0:F, 1 * C:2 * C] = w2 / cw
    cst[64:64 + F, 1 * C:2 * C] = 1.0 - np.square(w2) / np.square(cw)
    cst[0:F, 2 * C:3 * C] = 2.0 * w2 / np.square(cw)

    sel2 = np.zeros((33, 128), dtype=np.float32)
    sel2[0, 0:F] = 1.0
    sel2[32, 64:64 + F] = 1.0
    ones2 = np.zeros((124, 33), dtype=np.float32)
    ones2[0:F, 0] = 1.0
    ones2[64:124, 32] = 1.0
    sela = np.zeros((125, 128), dtype=np.float32)
    sela[124, :] = 1.0

    h = hidden1_weights.astype(np.float64)              # [2*C*F, OUT]
    h1 = h[:C * F].reshape(F, C, OUT) / math.sqrt(C)    # fold 2nd l2n of fv1
    h2 = h[C * F:].reshape(F, C, OUT)
    hds = np.concatenate([h1, h2], axis=1).reshape(F, 2 * C * OUT)
    hds = np.ascontiguousarray(hds, dtype=np.float32)

    in_maps = []
    for ci in range(NCORES):
        in_maps.append({
            "xg": np.ascontiguousarray(xgp[ci]),
            "xt": np.ascontiguousarray(xtp[ci]),
            "waug2": waug2,
            "cst": cst,
            "sel2": sel2,
            "ones2": ones2,
            "sela": sela,
            "hds": hds,
        })
    return in_maps


_CACHE = {}


def _get_nc():
    if "nc" not in _CACHE:
        _CACHE["nc"] = _build_nc()
    return _CACHE["nc"]


def kernel(reshaped_input, cluster_weights, covar_weights, cluster_biases,
           cluster_weights2, hidden1_weights, **_kw):
    in_maps = _host_prep(reshaped_input, cluster_weights, covar_weights,
                         cluster_biases, cluster_weights2, hidden1_weights)
    nc = _get_nc()
    res = run_bass_kernel_spmd(nc, in_maps, list(range(NCORES)))
    ys = [res.results[ci]["y"] for ci in range(NCORES)]
    return np.ascontiguousarray(np.concatenate(ys, axis=0), dtype=np.float32)


if __name__ == "__main__":
    rng = np.random.default_rng(0)
    fake = {
        "reshaped_input": rng.standard_normal((B * M, F), dtype=np.float32),
        "cluster_weights": rng.standard_normal((F, C)).astype(np.float32) * 0.13,
        "covar_weights": rng.standard_normal((F, C)).astype(np.float32) * 0.13,
        "cluster_biases": rng.standard_normal((C,)).astype(np.float32) * 0.13,
        "cluster_weights2": rng.standard_normal((1, F, C)).astype(np.float32) * 0.13,
        "hidden1_weights": rng.standard_normal((2 * C * F, OUT)).astype(np.float32) * 0.35,
    }
    out = kernel(**fake)
    print("kernel output", out.shape, out.dtype, np.abs(out).mean())
